# revision 30
# baseline (speedup 1.0000x reference)
"""Trainium2 Bass kernel for nn_AttentionModule (Bahdanau-style attention).

Reference computation (S=512, B=64, H=1024, F=2H):
    cat    = concat([hidden bcast to (S,B,H), encoder_states], -1)      [S,B,2H]
    scores = tanh(cat @ W_attn.T + b_attn) @ W_attn2.T + b_attn2        [S,B,1]
    attn   = softmax(scores[..., 0].T, axis=-1)                         [B,S]
    applied= einsum("bs,sbh->bh", attn, encoder_states)                 [B,H]
    out    = tanh(concat([decoder_out, applied], -1) @ W_comb.T + b_comb)

Sharding: data-parallel over B across 8 cores (8 batch rows per core).

Optimized structure (vs the bf16 baseline):
  - The dominant matmul T[f,s] = W1e @ enc (per b: [2048,1024]@[1024,512])
    runs in fp8e4 with perf_mode=DoubleRow: each instruction consumes two
    128-deep contraction chunks at once (~1.8x the bf16 rate).  W1e and
    W1h ship pre-scaled by 256 (fp8e4 subnormal range); consumers
    un-scale with activation scale=1/256.  The encoder ships twice: bf16
    [b,(kc p),s] for the attention apply, and fp8 pre-arranged to
    [b,p,(kc s)] (plain 2D DMA, 4KB rows) for the matmul.
  - Batch rows are processed in groups of 2 with b innermost so each
    DoubleRow weight load is reused (LDWEIGHTS has no FWL in DoubleRow
    mode).  PSUM budget (8 banks): psPre(1) closes after the preamble,
    then psT(4) + psSc(3: per-b score accumulator banks) + psOut(1).
  - The scores matmul (contract f: w2 . tanh) accumulates inline, one
    partial matmul per (ft,b) emitted one ft behind the main sweep.
  - hid @ W1h.T preamble and the final combine run operand-swapped
    (weights stationary, batch as the 8-wide moving operand) producing
    transposed outputs directly -- no PE transposes.  The decoder_out
    half of the final combine is emitted mid-sweep, off the critical
    tail.
  - DMA issue costs ~0.6us per descriptor on the issuing engine's queue,
    so transfers are batched (small constants concatenated host-side
    into one tensor) and split across the two HWDGE queues: critical
    loads (consts, w1h, w2t, enc_q) on SyncE, bulk/late loads (enc bf16,
    wct, attn bounces) on ScalarE.
  - softmax -> DRAM-bounce broadcast -> apply (VectorE mult+reduce over
    resident bf16 enc tiles), overlapping the next group's PE sweep.

Known pitfalls baked in:
  - small tensors ship fp32 and are cast on device (tiny bf16 rows get
    corrupted host->device); >=32B inner blocks for rearrange DMAs.
  - 16/32-bit matmul operand mixing rejected; fp8 pairs must both be fp8.
  - PSUM zero regions are 2KB: one accumulation group per bank region;
    sub-bank slices may emit start=True only on first touch.
  - vector.tensor_tensor_reduce breaks hardware execution (INTERNAL
    error) though CoreSim accepts it -- use tensor_tensor + reduce_sum.
"""

import numpy as np

S, B, H = 512, 64, 1024
F = 2 * H
NCORES = 8
BL = B // NCORES          # 8 batch rows per core
KH = H // 128             # 8 contraction chunks over H
KF = F // 128             # 16 feature tiles
GB = 2                    # batch rows per group (PSUM-bank limited)
NG = BL // GB             # 4 groups
WSCALE = 256.0            # fp8 weight pre-scale (power of 2)

_CACHE = {}


def _build(num_devices=NCORES):
    from contextlib import ExitStack

    import concourse.tile as tile
    from concourse import bacc, mybir

    f32 = mybir.dt.float32
    bf16 = mybir.dt.bfloat16
    fp8 = mybir.dt.float8e4
    AF = mybir.ActivationFunctionType
    ALU = mybir.AluOpType
    AX = mybir.AxisListType
    DR = mybir.MatmulPerfMode.DoubleRow

    nc = bacc.Bacc("TRN2", target_bir_lowering=False, debug=False,
                   num_devices=num_devices)

    # encoder copies, host-pre-arranged to [b, partition, kc*S+s] so the
    # loads are plain 2D DMAs with contiguous rows per partition (strided
    # rearrange DMAs cost ~us-scale descriptor generation on the queue).
    enc_t = nc.dram_tensor("enc_t", [BL, 128, KH * S], bf16,
                           kind="ExternalInput").ap()
    enc_q_d = nc.dram_tensor("enc_q", [BL, 128, KH * S], fp8,
                             kind="ExternalInput").ap()
    w1h = nc.dram_tensor("w1h", [H, F], fp8, kind="ExternalInput").ap()
    w2t = nc.dram_tensor("w2t", [H, F], fp8, kind="ExternalInput").ap()
    # wct halves pre-arranged to [half, partition, kc*H+h]
    wct = nc.dram_tensor("wct", [2, 128, KH * H], bf16,
                         kind="ExternalInput").ap()
    # smalls pre-arranged to [partition, k*BL+b] (hidT | decT | w2rep)
    smalls_d = nc.dram_tensor("smalls", [128, 32 * BL], f32,
                              kind="ExternalInput").ap()
    # bias = concat([b_attn * 256, b_comb], axis=1)
    bias_d = nc.dram_tensor("bias", [1, F + H], f32, kind="ExternalInput").ap()
    # outputs stay in the on-chip [partition, k*BL+b] layout; the host
    # un-arranges (keeps the tail DMAs to two fast contiguous writes).
    outT_d = nc.dram_tensor("outT", [128, 8 * BL], f32,
                            kind="ExternalOutput").ap()
    appT_d = nc.dram_tensor("appliedT", [128, KH * BL], f32,
                            kind="ExternalOutput").ap()

    with tile.TileContext(nc) as tc:
        with ExitStack() as ctx:
            consts = ctx.enter_context(tc.tile_pool(name="consts", bufs=1))
            encbf_pool = ctx.enter_context(tc.tile_pool(name="encbf", bufs=BL))
            encq_pool = ctx.enter_context(tc.tile_pool(name="encq", bufs=BL))
            w1h_pool = ctx.enter_context(tc.tile_pool(name="w1h", bufs=8))
            th_pool = ctx.enter_context(tc.tile_pool(name="th", bufs=10))
            attn_pool = ctx.enter_context(tc.tile_pool(name="attn", bufs=2))
            abc_pool = ctx.enter_context(tc.tile_pool(name="abc", bufs=2))
            scr_pool = ctx.enter_context(tc.tile_pool(name="scr", bufs=2))
            small_pool = ctx.enter_context(tc.tile_pool(name="small", bufs=8))
            dram_pool = ctx.enter_context(
                tc.tile_pool(name="dram", bufs=2, space="DRAM"))

            # ---- batched constants (one DMA each, SyncE queue) ------------
            ones8 = consts.tile([1, BL], bf16)
            nc.vector.memset(ones8[:], 1.0)
            smalls_32 = consts.tile([128, 32 * BL], f32)
            nc.sync.dma_start(smalls_32[:], smalls_d[:])
            bias_32 = consts.tile([1, F + H], f32)
            nc.sync.dma_start(bias_32[:], bias_d[:])

            hidT_q = consts.tile([128, KH * BL], fp8)
            nc.vector.tensor_copy(hidT_q[:], smalls_32[:, 0:KH * BL])
            decT_sb = consts.tile([128, KH * BL], bf16)
            nc.vector.tensor_copy(decT_sb[:],
                                  smalls_32[:, KH * BL:2 * KH * BL])
            w2rep_sb = consts.tile([128, KF * BL], bf16)
            nc.vector.tensor_copy(w2rep_sb[:],
                                  smalls_32[:, 2 * KH * BL:4 * KH * BL])
            b_attn_sb = consts.tile([1, F], bf16)
            nc.vector.tensor_copy(b_attn_sb[:], bias_32[:, 0:F])
            b_comb_sb = consts.tile([1, H], bf16)
            nc.vector.tensor_copy(b_comb_sb[:], bias_32[:, F:])

            # ---- critical-path loads (SyncE): w1h, w2t, first fp8 enc -----
            w1h_tiles = []
            for kc in range(KH):
                w1c = w1h_pool.tile([128, F], fp8, tag="w1h", name="w1c")
                nc.sync.dma_start(w1c[:], w1h[kc * 128:(kc + 1) * 128, :])
                w1h_tiles.append(w1c)

            w2t_sb = consts.tile([128, KH * F], fp8)
            for kc in range(KH):
                nc.sync.dma_start(w2t_sb[:, kc * F:(kc + 1) * F],
                                  w2t[kc * 128:(kc + 1) * 128, :])

            def load_encq(b):
                t = encq_pool.tile([128, KH * S], fp8, tag="encq",
                                   name=f"encq{b}")
                nc.sync.dma_start(t[:], enc_q_d[b])
                return t

            def load_encbf(b):
                # bulk load on the ScalarE HWDGE queue
                t = encbf_pool.tile([128, KH * S], bf16, tag="encbf",
                                    name=f"encbf{b}")
                nc.scalar.dma_start(t[:], enc_t[b])
                return t

            et_q = {b: load_encq(b) for b in range(2 * GB)}
            et_bf = {}

            # ---- preamble: hidbT[f, b] = (hidden @ W1h.T + b_attn)^T ------
            # operand-swapped: W1h chunks stationary (fp8, x256), batch (8)
            # moving; output lands transposed (f on partitions), un-scaled
            # by the copy-out.  ps_pre is one PSUM bank with 16 sub-bank
            # slices: start=True only on first touch (2KB zero region).
            hidbT_sb = consts.tile([128, KF * BL], f32)
            with tc.tile_pool(name="psPre", bufs=1, space="PSUM") as psPre_pool:
                ps_pre = psPre_pool.tile([128, KF * BL], f32, tag="pre",
                                         name="pspre")
                for kc in range(KH):
                    w1c = w1h_tiles[kc]
                    for ft in range(KF):
                        nc.tensor.matmul(
                            ps_pre[:, ft * BL:(ft + 1) * BL],
                            w1c[:, ft * 128:(ft + 1) * 128],
                            hidT_q[:, kc * BL:(kc + 1) * BL],
                            start=(kc == 0 and ft == 0), stop=False,
                            skip_group_check=True)
                for ft in range(KF):
                    nc.tensor.matmul(
                        ps_pre[:, ft * BL:(ft + 1) * BL],
                        b_attn_sb[:, ft * 128:(ft + 1) * 128],
                        ones8[:],
                        start=False, stop=(ft == KF - 1),
                        skip_group_check=True)
                # un-scale the x256 preamble on ScalarE (VectorE stays free)
                nc.scalar.activation(hidbT_sb[:], ps_pre[:], AF.Copy,
                                     scale=1.0 / WSCALE)

            # ---- shared applied-attention accumulators --------------------
            appT_sb = consts.tile([128, KH * BL], f32)
            appT_bf = consts.tile([128, KH * BL], bf16)

            def emit_scores_mm(psc, ft, b):
                # psc (all 8 rows identical) += w2(ft) . th(ft,b)
                nc.tensor.matmul(
                    psc[:],
                    w2rep_sb[:, ft * BL:(ft + 1) * BL],
                    th_tiles[(ft, b)][:],
                    start=(ft == 0), stop=(ft == KF - 1))
                del th_tiles[(ft, b)]

            def emit_softmax_apply(psc, b, scalar_reduce=False):
                negmax = small_pool.tile([BL, 1], f32, tag="negmax",
                                         name="negmax")
                nc.vector.reduce_max(negmax[:], psc[:], axis=AX.X,
                                     negate=True)
                attn = attn_pool.tile([BL, S], bf16, tag="attn", name="attn")
                sumexp = small_pool.tile([BL, 1], f32, tag="sumexp",
                                         name="sumexp")
                nc.scalar.activation(attn[:], psc[:], AF.Exp,
                                     bias=negmax[:], scale=1.0,
                                     accum_out=sumexp[:])
                recip = small_pool.tile([BL, 1], f32, tag="recip",
                                        name="recip")
                nc.vector.reciprocal(recip[:], sumexp[:])
                # only row 0 is broadcast -- normalize just that row
                nc.vector.tensor_scalar_mul(attn[0:1, :], attn[0:1, :],
                                            recip[0:1, :])

                # broadcast attn row across 128 partitions via DRAM bounce
                attn_dr = dram_pool.tile([1, S], bf16, tag="attn_dr",
                                         name="attn_dr")
                nc.scalar.dma_start(attn_dr[:], attn[0:1, :])
                abc = abc_pool.tile([128, S], bf16, tag="abc", name="abc")
                nc.scalar.dma_start(abc[:],
                                    attn_dr[0:1, :].to_broadcast((128, S)))

                et = et_bf[b]
                for kc in range(KH):
                    scr = scr_pool.tile([128, S], bf16, tag="scr", name="scr")
                    nc.vector.tensor_tensor(
                        out=scr[:], in0=et[:, kc * S:(kc + 1) * S],
                        in1=abc[:], op=ALU.mult)
                    acol = appT_sb[:, kc * BL + b: kc * BL + b + 1]
                    if scalar_reduce:
                        # tail: reduce on ScalarE (activation accumulator)
                        # so VectorE's mults and the reduces run in parallel
                        junk = scr_pool.tile([128, S], bf16, tag="scr",
                                             name="junk")
                        nc.scalar.activation(junk[:], scr[:], AF.Copy,
                                             accum_out=acol)
                    else:
                        nc.vector.reduce_sum(acol, scr[:], axis=AX.X)
                nc.vector.tensor_copy(
                    appT_bf.rearrange("p (k b) -> p k b", b=BL)[:, :, b],
                    appT_sb.rearrange("p (k b) -> p k b", b=BL)[:, :, b])

            # ---- main sweep: NG groups of GB batch rows -------------------
            psT_pool = ctx.enter_context(
                tc.tile_pool(name="psT", bufs=4, space="PSUM"))
            psSc_pool = ctx.enter_context(
                tc.tile_pool(name="psSc", bufs=3, space="PSUM"))

            th_tiles = {}
            w2t_3d = w2t_sb.rearrange("p (k f) -> p k f", f=F)
            pending = []          # (ft, b) scores matmuls delayed one ft
            ps_out = None
            wct_dec = wct_app = None

            for g in range(NG):
                bs = range(g * GB, (g + 1) * GB)
                psc = {b: psSc_pool.tile([BL, S], f32, tag="psc",
                                         name=f"psc{b%GB}") for b in bs}
                psT = {}
                for ft in range(KF):
                    for kp in range(KH // 2):
                        # DoubleRow pair: contraction chunks 2kp, 2kp+1
                        lhs = w2t_3d[:, 2 * kp:2 * kp + 2,
                                     ft * 128:(ft + 1) * 128]
                        for b in bs:
                            if kp == 0:
                                psT[b] = psT_pool.tile(
                                    [128, S], f32, tag="pT", name=f"pT{b%GB}")
                            eq3 = et_q[b].rearrange("p (k s) -> p k s", s=S)
                            nc.tensor.matmul(
                                psT[b][:], lhs,
                                eq3[:, 2 * kp:2 * kp + 2, :],
                                start=(kp == 0), stop=(kp == KH // 2 - 1),
                                perf_mode=DR)
                    for b in bs:
                        t = th_pool.tile([128, S], bf16, tag="tanh",
                                         name="tanh")
                        nc.scalar.activation(
                            t[:], psT[b][:], AF.Tanh,
                            bias=hidbT_sb[:, ft * BL + b: ft * BL + b + 1],
                            scale=1.0 / WSCALE)
                        th_tiles[(ft, b)] = t
                    # scores run one ft behind so tanh is off the PE path
                    for (pft, pb) in pending:
                        emit_scores_mm(psc[pb], pft, pb)
                    pending = [(ft, b) for b in bs]

                    if ft == 0:
                        # prefetch the group-after-next's fp8 enc
                        for b in range((g + 2) * GB,
                                       min((g + 3) * GB, BL)):
                            et_q[b] = load_encq(b)
                    if ft == 6:
                        # this group's bf16 enc (needed at group end);
                        # deferred so it stays off the critical startup DMAs
                        for b in bs:
                            et_bf[b] = load_encbf(b)
                    if g == 1 and ft == 8:
                        # wct decoder-half block load (ScalarE queue)
                        wct_dec = consts.tile([128, KH * H], bf16)
                        nc.scalar.dma_start(wct_dec[:], wct[0])
                    if g == 2 and ft == 4:
                        # final combine, decoder_out half: out^T[f,b]
                        # accumulates in one PSUM bank with 8 sub-bank
                        # slices (single-start rule, see ps_pre).
                        psOut_pool = ctx.enter_context(
                            tc.tile_pool(name="psOut", bufs=1, space="PSUM"))
                        ps_out = psOut_pool.tile([128, 8 * BL], f32,
                                                 tag="out", name="psout")
                        for kc in range(KH):
                            for fc in range(8):
                                nc.tensor.matmul(
                                    ps_out[:, fc * BL:(fc + 1) * BL],
                                    wct_dec[:, kc * H + fc * 128:
                                            kc * H + (fc + 1) * 128],
                                    decT_sb[:, kc * BL:(kc + 1) * BL],
                                    start=(kc == 0 and fc == 0), stop=False,
                                    skip_group_check=True)
                    if g == 2 and ft == 8:
                        # wct applied-half block load (ScalarE queue)
                        wct_app = consts.tile([128, KH * H], bf16)
                        nc.scalar.dma_start(wct_app[:], wct[1])

                # last ft's scores, then softmax/apply for this group
                # (the V/S/DMA chain overlaps the next group's PE sweep)
                for (pft, pb) in pending:
                    emit_scores_mm(psc[pb], pft, pb)
                pending = []
                for b in bs:
                    emit_softmax_apply(psc[b], b, scalar_reduce=(g == NG - 1))

            # ---- final combine, applied half + bias + tanh ----------------
            for kc in range(KH):
                for fc in range(8):
                    nc.tensor.matmul(
                        ps_out[:, fc * BL:(fc + 1) * BL],
                        wct_app[:, kc * H + fc * 128:
                                kc * H + (fc + 1) * 128],
                        appT_bf[:, kc * BL:(kc + 1) * BL],
                        start=False, stop=False,
                        skip_group_check=True)
            for fc in range(8):
                nc.tensor.matmul(
                    ps_out[:, fc * BL:(fc + 1) * BL],
                    b_comb_sb[:, fc * 128:(fc + 1) * 128],
                    ones8[:],
                    start=False, stop=(fc == 7),
                    skip_group_check=True)

            outT_sb = consts.tile([128, 8 * BL], f32)
            nc.scalar.activation(outT_sb[:], ps_out[:], AF.Tanh)
            nc.sync.dma_start(outT_d[:], outT_sb[:])
            nc.sync.dma_start(appT_d[:], appT_sb[:])

    nc.compile()
    return nc


def _get_nc():
    if "nc" not in _CACHE:
        _CACHE["nc"] = _build()
    return _CACHE["nc"]


def make_in_maps(inputs):
    import ml_dtypes
    bf = ml_dtypes.bfloat16
    f8 = ml_dtypes.float8_e4m3fn

    inp = {k: np.asarray(v, dtype=np.float32) for k, v in inputs.items()}
    hidden = inp["hidden"]
    decoder_out = inp["decoder_out"]
    encoder_states = inp["encoder_states"]
    W_attn = inp["W_attn"]
    b_attn = inp["b_attn"]
    W_attn2 = inp["W_attn2"]
    W_comb = inp["W_comb"]
    b_comb = inp["b_comb"]
    # b_attn2 shifts every score equally -> softmax-invariant, unused.

    watT = np.ascontiguousarray(W_attn.T)                     # [2H, 2H]
    w1h = np.ascontiguousarray(watT[:H] * WSCALE).astype(f8)  # hidden-half
    w2t = np.ascontiguousarray(watT[H:] * WSCALE).astype(f8)  # encoder-half
    wct = np.ascontiguousarray(
        W_comb.T.reshape(2, KH, 128, H).transpose(0, 2, 1, 3)
        .reshape(2, 128, KH * H)).astype(bf)
    w2rep = np.repeat(W_attn2.reshape(F, 1), BL, axis=1)
    bias = np.ascontiguousarray(np.concatenate(
        [b_attn.reshape(1, F) * WSCALE, b_comb.reshape(1, H)],
        axis=1).astype(np.float32))

    in_maps = []
    for c in range(NCORES):
        sl = slice(c * BL, (c + 1) * BL)
        enc_pc = np.ascontiguousarray(
            encoder_states[:, sl, :].transpose(1, 2, 0)
            .reshape(BL, KH, 128, S).transpose(0, 2, 1, 3)
            .reshape(BL, 128, KH * S))                        # [BL, p, k*s]
        smalls = np.concatenate(
            [hidden[sl].T, decoder_out[sl].T, w2rep],
            axis=0).astype(np.float32)                        # [4H, BL]
        smalls = np.ascontiguousarray(
            smalls.reshape(32, 128, BL).transpose(1, 0, 2)
            .reshape(128, 32 * BL))                           # [p, k*BL+b]
        in_maps.append({
            "enc_t": enc_pc.astype(bf),
            "enc_q": enc_pc.astype(f8),
            "w1h": w1h,
            "w2t": w2t,
            "wct": wct,
            "smalls": smalls,
            "bias": bias,
        })
    return in_maps


def kernel(**inputs):
    from concourse.bass_utils import run_bass_kernel_spmd

    in_maps = make_in_maps(inputs)
    nc = _get_nc()
    res = run_bass_kernel_spmd(nc, in_maps, list(range(NCORES)))
    def unarr(a):
        # [128, k*BL+b] -> [k*128+p, b] -> [b, kp]
        return a.reshape(128, -1, BL).transpose(1, 0, 2).reshape(H, BL).T

    out = np.concatenate(
        [unarr(res.results[c]["outT"]) for c in range(NCORES)], axis=0)
    applied = np.concatenate(
        [unarr(res.results[c]["appliedT"]) for c in range(NCORES)], axis=0)
    return out.astype(np.float32), applied.astype(np.float32)


# revision 33
# speedup vs baseline: 1.0500x; 1.0500x over previous
"""Trainium2 Bass kernel for nn_AttentionModule (Bahdanau-style attention).

Reference computation (S=512, B=64, H=1024, F=2H):
    cat    = concat([hidden bcast to (S,B,H), encoder_states], -1)      [S,B,2H]
    scores = tanh(cat @ W_attn.T + b_attn) @ W_attn2.T + b_attn2        [S,B,1]
    attn   = softmax(scores[..., 0].T, axis=-1)                         [B,S]
    applied= einsum("bs,sbh->bh", attn, encoder_states)                 [B,H]
    out    = tanh(concat([decoder_out, applied], -1) @ W_comb.T + b_comb)

Sharding: data-parallel over B across 8 cores (8 batch rows per core).

Optimized structure (vs the bf16 baseline):
  - The dominant matmul T[f,s] = W1e @ enc (per b: [2048,1024]@[1024,512])
    runs in fp8e4 with perf_mode=DoubleRow: each instruction consumes two
    128-deep contraction chunks at once (~1.8x the bf16 rate).  W1e and
    W1h ship pre-scaled by 256 (fp8e4 subnormal range); consumers
    un-scale with activation scale=1/256.  The encoder ships twice: bf16
    [b,(kc p),s] for the attention apply, and fp8 pre-arranged to
    [b,p,(kc s)] (plain 2D DMA, 4KB rows) for the matmul.
  - Batch rows are processed in groups of 2 with b innermost so each
    DoubleRow weight load is reused (LDWEIGHTS has no FWL in DoubleRow
    mode).  PSUM budget (8 banks): psPre(1) closes after the preamble,
    then psT(4) + psSc(3: per-b score accumulator banks) + psOut(1).
  - The scores matmul (contract f: w2 . tanh) accumulates inline, one
    partial matmul per (ft,b) emitted one ft behind the main sweep.
  - hid @ W1h.T preamble and the final combine run operand-swapped
    (weights stationary, batch as the 8-wide moving operand) producing
    transposed outputs directly -- no PE transposes.  The decoder_out
    half of the final combine is emitted mid-sweep, off the critical
    tail.
  - DMA issue costs ~0.6us per descriptor on the issuing engine's queue,
    so transfers are batched (small constants concatenated host-side
    into one tensor) and split across the two HWDGE queues: critical
    loads (consts, w1h, w2t, enc_q) on SyncE, bulk/late loads (enc bf16,
    wct, attn bounces) on ScalarE.
  - softmax -> DRAM-bounce broadcast -> apply (VectorE mult+reduce over
    resident bf16 enc tiles), overlapping the next group's PE sweep.

Known pitfalls baked in:
  - small tensors ship fp32 and are cast on device (tiny bf16 rows get
    corrupted host->device); >=32B inner blocks for rearrange DMAs.
  - 16/32-bit matmul operand mixing rejected; fp8 pairs must both be fp8.
  - PSUM zero regions are 2KB: one accumulation group per bank region;
    sub-bank slices may emit start=True only on first touch.
  - vector.tensor_tensor_reduce breaks hardware execution (INTERNAL
    error) though CoreSim accepts it -- use tensor_tensor + reduce_sum.
"""

import numpy as np

S, B, H = 512, 64, 1024
F = 2 * H
NCORES = 8
BL = B // NCORES          # 8 batch rows per core
KH = H // 128             # 8 contraction chunks over H
KF = F // 128             # 16 feature tiles
GB = 2                    # batch rows per group (PSUM-bank limited)
NG = BL // GB             # 4 groups
WSCALE = 256.0            # fp8 weight pre-scale (power of 2)

_CACHE = {}


def _build(num_devices=NCORES):
    from contextlib import ExitStack

    import concourse.tile as tile
    from concourse import bacc, mybir

    f32 = mybir.dt.float32
    bf16 = mybir.dt.bfloat16
    fp8 = mybir.dt.float8e4
    AF = mybir.ActivationFunctionType
    ALU = mybir.AluOpType
    AX = mybir.AxisListType
    DR = mybir.MatmulPerfMode.DoubleRow

    nc = bacc.Bacc("TRN2", target_bir_lowering=False, debug=False,
                   num_devices=num_devices)

    # encoder copies, host-pre-arranged to [b, partition, kc*S+s] so the
    # loads are plain 2D DMAs with contiguous rows per partition (strided
    # rearrange DMAs cost ~us-scale descriptor generation on the queue).
    enc_t = nc.dram_tensor("enc_t", [NG, 128, GB * KH * S], bf16,
                           kind="ExternalInput").ap()
    enc_q_d = nc.dram_tensor("enc_q", [NG, 128, GB * KH * S], fp8,
                             kind="ExternalInput").ap()
    # w1h/w2t pre-arranged to [partition, kc*F+f]: one >=1MB DMA each
    # (transfers under ~1MB are descriptor-dominated, ~100 GB/s).
    w1h = nc.dram_tensor("w1h", [128, KH * F], fp8,
                         kind="ExternalInput").ap()
    w2t = nc.dram_tensor("w2t", [128, KH * F], fp8,
                         kind="ExternalInput").ap()
    # wct halves pre-arranged to [half, partition, kc*H+h]
    wct = nc.dram_tensor("wct", [2, 128, KH * H], bf16,
                         kind="ExternalInput").ap()
    # smalls pre-arranged to [partition, k*BL+b] (hidT | decT | w2rep)
    smalls_d = nc.dram_tensor("smalls", [128, 32 * BL], f32,
                              kind="ExternalInput").ap()
    # bias = concat([b_attn * 256, b_comb], axis=1)
    bias_d = nc.dram_tensor("bias", [1, F + H], f32, kind="ExternalInput").ap()
    # outputs stay in the on-chip [partition, k*BL+b] layout; the host
    # un-arranges (keeps the tail DMAs to two fast contiguous writes).
    outT_d = nc.dram_tensor("outT", [128, 8 * BL], f32,
                            kind="ExternalOutput").ap()
    appT_d = nc.dram_tensor("appliedT", [128, KH * BL], f32,
                            kind="ExternalOutput").ap()

    with tile.TileContext(nc) as tc:
        with ExitStack() as ctx:
            consts = ctx.enter_context(tc.tile_pool(name="consts", bufs=1))
            encbf_pool = ctx.enter_context(tc.tile_pool(name="encbf", bufs=NG))
            encq_pool = ctx.enter_context(tc.tile_pool(name="encq", bufs=NG))
            th_pool = ctx.enter_context(tc.tile_pool(name="th", bufs=10))
            attn_pool = ctx.enter_context(tc.tile_pool(name="attn", bufs=2))
            abc_pool = ctx.enter_context(tc.tile_pool(name="abc", bufs=2))
            scr_pool = ctx.enter_context(tc.tile_pool(name="scr", bufs=4))
            junk_pool = ctx.enter_context(tc.tile_pool(name="junk", bufs=2))
            small_pool = ctx.enter_context(tc.tile_pool(name="small", bufs=8))
            dram_pool = ctx.enter_context(
                tc.tile_pool(name="dram", bufs=2, space="DRAM"))

            # ---- batched constants (one DMA each, SyncE queue) ------------
            ones8 = consts.tile([1, BL], bf16)
            nc.vector.memset(ones8[:], 1.0)
            smalls_32 = consts.tile([128, 32 * BL], f32)
            nc.sync.dma_start(smalls_32[:], smalls_d[:])
            bias_32 = consts.tile([1, F + H], f32)
            nc.sync.dma_start(bias_32[:], bias_d[:])

            hidT_q = consts.tile([128, KH * BL], fp8)
            nc.vector.tensor_copy(hidT_q[:], smalls_32[:, 0:KH * BL])
            decT_sb = consts.tile([128, KH * BL], bf16)
            nc.vector.tensor_copy(decT_sb[:],
                                  smalls_32[:, KH * BL:2 * KH * BL])
            w2rep_sb = consts.tile([128, KF * BL], bf16)
            nc.vector.tensor_copy(w2rep_sb[:],
                                  smalls_32[:, 2 * KH * BL:4 * KH * BL])
            b_attn_sb = consts.tile([1, F], bf16)
            nc.vector.tensor_copy(b_attn_sb[:], bias_32[:, 0:F])
            b_comb_sb = consts.tile([1, H], bf16)
            nc.vector.tensor_copy(b_comb_sb[:], bias_32[:, F:])

            # ---- critical-path loads: one big DMA each ---------------
            # w1h on the ScalarE queue (runs parallel to SyncE's w2t/enc).
            w1h_sb = consts.tile([128, KH * F], fp8)
            nc.scalar.dma_start(w1h_sb[:], w1h[:])

            w2t_sb = consts.tile([128, KH * F], fp8)
            nc.sync.dma_start(w2t_sb[:], w2t[:])

            def load_encq(g):
                # one 1MB DMA per group (both batch rows)
                t = encq_pool.tile([128, GB * KH * S], fp8, tag="encq",
                                   name=f"encq{g}")
                nc.sync.dma_start(t[:], enc_q_d[g])
                return t

            def load_encbf(g):
                # one 2MB DMA per group on the ScalarE queue
                t = encbf_pool.tile([128, GB * KH * S], bf16, tag="encbf",
                                    name=f"encbf{g}")
                nc.scalar.dma_start(t[:], enc_t[g])
                return t

            eq_g = {g: load_encq(g) for g in range(2)}
            ebf_g = {}

            # ---- preamble: hidbT[f, b] = (hidden @ W1h.T + b_attn)^T ------
            # operand-swapped: W1h chunks stationary (fp8, x256), batch (8)
            # moving; output lands transposed (f on partitions), un-scaled
            # by the copy-out.  ps_pre is one PSUM bank with 16 sub-bank
            # slices: start=True only on first touch (2KB zero region).
            hidbT_sb = consts.tile([128, KF * BL], f32)
            with tc.tile_pool(name="psPre", bufs=1, space="PSUM") as psPre_pool:
                ps_pre = psPre_pool.tile([128, KF * BL], f32, tag="pre",
                                         name="pspre")
                for kc in range(KH):
                    for ft in range(KF):
                        nc.tensor.matmul(
                            ps_pre[:, ft * BL:(ft + 1) * BL],
                            w1h_sb[:, kc * F + ft * 128:
                                   kc * F + (ft + 1) * 128],
                            hidT_q[:, kc * BL:(kc + 1) * BL],
                            start=(kc == 0 and ft == 0), stop=False,
                            skip_group_check=True)
                for ft in range(KF):
                    nc.tensor.matmul(
                        ps_pre[:, ft * BL:(ft + 1) * BL],
                        b_attn_sb[:, ft * 128:(ft + 1) * 128],
                        ones8[:],
                        start=False, stop=(ft == KF - 1),
                        skip_group_check=True)
                # un-scale the x256 preamble on ScalarE (VectorE stays free)
                nc.scalar.activation(hidbT_sb[:], ps_pre[:], AF.Copy,
                                     scale=1.0 / WSCALE)

            # ---- shared applied-attention accumulators --------------------
            appT_sb = consts.tile([128, KH * BL], f32)
            appT_bf = consts.tile([128, KH * BL], bf16)

            def emit_scores_mm(psc, ft, b):
                # psc (all 8 rows identical) += w2(ft) . th(ft,b)
                nc.tensor.matmul(
                    psc[:],
                    w2rep_sb[:, ft * BL:(ft + 1) * BL],
                    th_tiles[(ft, b)][:],
                    start=(ft == 0), stop=(ft == KF - 1))
                del th_tiles[(ft, b)]

            def emit_softmax_apply(psc, b, scalar_reduce=False):
                negmax = small_pool.tile([BL, 1], f32, tag="negmax",
                                         name="negmax")
                nc.vector.reduce_max(negmax[:], psc[:], axis=AX.X,
                                     negate=True)
                attn = attn_pool.tile([BL, S], bf16, tag="attn", name="attn")
                sumexp = small_pool.tile([BL, 1], f32, tag="sumexp",
                                         name="sumexp")
                nc.scalar.activation(attn[:], psc[:], AF.Exp,
                                     bias=negmax[:], scale=1.0,
                                     accum_out=sumexp[:])
                recip = small_pool.tile([BL, 1], f32, tag="recip",
                                        name="recip")
                nc.vector.reciprocal(recip[:], sumexp[:])
                # only row 0 is broadcast -- normalize just that row
                nc.vector.tensor_scalar_mul(attn[0:1, :], attn[0:1, :],
                                            recip[0:1, :])

                # broadcast attn row across 128 partitions via DRAM bounce
                attn_dr = dram_pool.tile([1, S], bf16, tag="attn_dr",
                                         name="attn_dr")
                nc.scalar.dma_start(attn_dr[:], attn[0:1, :])
                abc = abc_pool.tile([128, S], bf16, tag="abc", name="abc")
                nc.scalar.dma_start(abc[:],
                                    attn_dr[0:1, :].to_broadcast((128, S)))

                et = ebf_g[b // GB]
                boff = (b % GB) * KH * S
                for kc in range(KH):
                    scr = scr_pool.tile([128, S], bf16, tag="scr", name="scr")
                    nc.vector.tensor_tensor(
                        out=scr[:],
                        in0=et[:, boff + kc * S:boff + (kc + 1) * S],
                        in1=abc[:], op=ALU.mult)
                    acol = appT_sb[:, kc * BL + b: kc * BL + b + 1]
                    if scalar_reduce:
                        # tail: reduce on ScalarE (activation accumulator)
                        # so VectorE's mults and the reduces run in parallel
                        junk = junk_pool.tile([128, S], bf16, tag="junk",
                                              name="junk")
                        nc.scalar.activation(junk[:], scr[:], AF.Copy,
                                             accum_out=acol)
                    else:
                        nc.vector.reduce_sum(acol, scr[:], axis=AX.X)
                nc.vector.tensor_copy(
                    appT_bf.rearrange("p (k b) -> p k b", b=BL)[:, :, b],
                    appT_sb.rearrange("p (k b) -> p k b", b=BL)[:, :, b])

            # ---- main sweep: NG groups of GB batch rows -------------------
            psT_pool = ctx.enter_context(
                tc.tile_pool(name="psT", bufs=4, space="PSUM"))
            psSc_pool = ctx.enter_context(
                tc.tile_pool(name="psSc", bufs=3, space="PSUM"))

            th_tiles = {}
            w2t_3d = w2t_sb.rearrange("p (k f) -> p k f", f=F)
            pending = []          # (ft, b) scores matmuls delayed one ft
            ps_out = None
            wct_dec = wct_app = None

            for g in range(NG):
                bs = range(g * GB, (g + 1) * GB)
                psc = {b: psSc_pool.tile([BL, S], f32, tag="psc",
                                         name=f"psc{b%GB}") for b in bs}
                psT = {}
                for ft in range(KF):
                    for kp in range(KH // 2):
                        # DoubleRow pair: contraction chunks 2kp, 2kp+1
                        lhs = w2t_3d[:, 2 * kp:2 * kp + 2,
                                     ft * 128:(ft + 1) * 128]
                        eq4 = eq_g[g].rearrange(
                            "p (bb k s) -> p bb k s", k=KH, s=S)
                        for b in bs:
                            if kp == 0:
                                psT[b] = psT_pool.tile(
                                    [128, S], f32, tag="pT", name=f"pT{b%GB}")
                            nc.tensor.matmul(
                                psT[b][:], lhs,
                                eq4[:, b % GB, 2 * kp:2 * kp + 2, :],
                                start=(kp == 0), stop=(kp == KH // 2 - 1),
                                perf_mode=DR)
                    for b in bs:
                        t = th_pool.tile([128, S], bf16, tag="tanh",
                                         name="tanh")
                        nc.scalar.activation(
                            t[:], psT[b][:], AF.Tanh,
                            bias=hidbT_sb[:, ft * BL + b: ft * BL + b + 1],
                            scale=1.0 / WSCALE)
                        th_tiles[(ft, b)] = t
                    # scores run one ft behind so tanh is off the PE path
                    for (pft, pb) in pending:
                        emit_scores_mm(psc[pb], pft, pb)
                    pending = [(ft, b) for b in bs]

                    if ft == 0 and g + 2 < NG:
                        # prefetch the group-after-next's fp8 enc
                        eq_g[g + 2] = load_encq(g + 2)
                    if ft == 6:
                        # this group's bf16 enc (needed at group end);
                        # deferred so it stays off the critical startup DMAs
                        ebf_g[g] = load_encbf(g)
                    if g == 1 and ft == 8:
                        # wct decoder-half block load (ScalarE queue)
                        wct_dec = consts.tile([128, KH * H], bf16)
                        nc.scalar.dma_start(wct_dec[:], wct[0])
                    if g == 2 and ft == 4:
                        # final combine, decoder_out half: out^T[f,b]
                        # accumulates in one PSUM bank with 8 sub-bank
                        # slices (single-start rule, see ps_pre).
                        psOut_pool = ctx.enter_context(
                            tc.tile_pool(name="psOut", bufs=1, space="PSUM"))
                        ps_out = psOut_pool.tile([128, 8 * BL], f32,
                                                 tag="out", name="psout")
                        for kc in range(KH):
                            for fc in range(8):
                                nc.tensor.matmul(
                                    ps_out[:, fc * BL:(fc + 1) * BL],
                                    wct_dec[:, kc * H + fc * 128:
                                            kc * H + (fc + 1) * 128],
                                    decT_sb[:, kc * BL:(kc + 1) * BL],
                                    start=(kc == 0 and fc == 0), stop=False,
                                    skip_group_check=True)
                    if g == 2 and ft == 8:
                        # wct applied-half block load (ScalarE queue)
                        wct_app = consts.tile([128, KH * H], bf16)
                        nc.scalar.dma_start(wct_app[:], wct[1])

                # last ft's scores, then softmax/apply for this group
                # (the V/S/DMA chain overlaps the next group's PE sweep)
                for (pft, pb) in pending:
                    emit_scores_mm(psc[pb], pft, pb)
                pending = []
                for b in bs:
                    emit_softmax_apply(psc[b], b,
                                       scalar_reduce=(b == BL - 1))

            # ---- final combine, applied half + bias + tanh ----------------
            for kc in range(KH):
                for fc in range(8):
                    nc.tensor.matmul(
                        ps_out[:, fc * BL:(fc + 1) * BL],
                        wct_app[:, kc * H + fc * 128:
                                kc * H + (fc + 1) * 128],
                        appT_bf[:, kc * BL:(kc + 1) * BL],
                        start=False, stop=False,
                        skip_group_check=True)
            for fc in range(8):
                nc.tensor.matmul(
                    ps_out[:, fc * BL:(fc + 1) * BL],
                    b_comb_sb[:, fc * 128:(fc + 1) * 128],
                    ones8[:],
                    start=False, stop=(fc == 7),
                    skip_group_check=True)

            outT_sb = consts.tile([128, 8 * BL], f32)
            nc.scalar.activation(outT_sb[:], ps_out[:], AF.Tanh)
            nc.sync.dma_start(outT_d[:], outT_sb[:])
            nc.sync.dma_start(appT_d[:], appT_sb[:])

    nc.compile()
    return nc


def _get_nc():
    if "nc" not in _CACHE:
        _CACHE["nc"] = _build()
    return _CACHE["nc"]


def make_in_maps(inputs):
    import ml_dtypes
    bf = ml_dtypes.bfloat16
    f8 = ml_dtypes.float8_e4m3fn

    inp = {k: np.asarray(v, dtype=np.float32) for k, v in inputs.items()}
    hidden = inp["hidden"]
    decoder_out = inp["decoder_out"]
    encoder_states = inp["encoder_states"]
    W_attn = inp["W_attn"]
    b_attn = inp["b_attn"]
    W_attn2 = inp["W_attn2"]
    W_comb = inp["W_comb"]
    b_comb = inp["b_comb"]
    # b_attn2 shifts every score equally -> softmax-invariant, unused.

    watT = np.ascontiguousarray(W_attn.T)                     # [2H, 2H]

    def packw(w):
        # [H, F] -> [partition, kc*F+f]
        return np.ascontiguousarray(
            w.reshape(KH, 128, F).transpose(1, 0, 2).reshape(128, KH * F))

    w1h = packw(watT[:H] * WSCALE).astype(f8)                 # hidden-half
    w2t = packw(watT[H:] * WSCALE).astype(f8)                 # encoder-half
    wct = np.ascontiguousarray(
        W_comb.T.reshape(2, KH, 128, H).transpose(0, 2, 1, 3)
        .reshape(2, 128, KH * H)).astype(bf)
    w2rep = np.repeat(W_attn2.reshape(F, 1), BL, axis=1)
    bias = np.ascontiguousarray(np.concatenate(
        [b_attn.reshape(1, F) * WSCALE, b_comb.reshape(1, H)],
        axis=1).astype(np.float32))

    in_maps = []
    for c in range(NCORES):
        sl = slice(c * BL, (c + 1) * BL)
        enc_pc = np.ascontiguousarray(
            encoder_states[:, sl, :].transpose(1, 2, 0)
            .reshape(NG, GB, KH, 128, S).transpose(0, 3, 1, 2, 4)
            .reshape(NG, 128, GB * KH * S))                   # [g, p, b*k*s]
        smalls = np.concatenate(
            [hidden[sl].T, decoder_out[sl].T, w2rep],
            axis=0).astype(np.float32)                        # [4H, BL]
        smalls = np.ascontiguousarray(
            smalls.reshape(32, 128, BL).transpose(1, 0, 2)
            .reshape(128, 32 * BL))                           # [p, k*BL+b]
        in_maps.append({
            "enc_t": enc_pc.astype(bf),
            "enc_q": enc_pc.astype(f8),
            "w1h": w1h,
            "w2t": w2t,
            "wct": wct,
            "smalls": smalls,
            "bias": bias,
        })
    return in_maps


def kernel(**inputs):
    from concourse.bass_utils import run_bass_kernel_spmd

    in_maps = make_in_maps(inputs)
    nc = _get_nc()
    res = run_bass_kernel_spmd(nc, in_maps, list(range(NCORES)))
    def unarr(a):
        # [128, k*BL+b] -> [k*128+p, b] -> [b, kp]
        return a.reshape(128, -1, BL).transpose(1, 0, 2).reshape(H, BL).T

    out = np.concatenate(
        [unarr(res.results[c]["outT"]) for c in range(NCORES)], axis=0)
    applied = np.concatenate(
        [unarr(res.results[c]["appliedT"]) for c in range(NCORES)], axis=0)
    return out.astype(np.float32), applied.astype(np.float32)
